# revision 13
# baseline (speedup 1.0000x reference)
"""Fused dense-transformer block for Trainium2 (Bass/Tile), 8-core data-parallel.

Per batch row b of x[16, 2048, 512]:
  LayerNorm -> Q/K/V proj -> softmax(Q K^T / sqrt(H)) V -> quickGELU MLP(512->1024->1) -> [2048]

Sharding: batch dim 16 -> 8 cores x 2 batches each. No collectives.

Layout strategy (per core, per batch):
  - LN in natural layout [tok, h] (free-dim stats via bn_stats), then PE-transpose
    the normalized activations to z^T [h, tok].
  - Q^T, K^T computed in transposed layout [h_out, tok]; V in natural [tok, h].
  - Scores computed directly TRANSPOSED: S^T[key, q] = (K z-weights) so softmax
    exp happens on ACT reading PSUM, writing P^T[key, q] straight to SBUF --
    no transposes of the big [2048, 2048] attention matrix.
  - Row sums via ones-vector matmul (contraction over keys = partitions),
    reciprocal on DVE, broadcast via gpsimd, applied while evacuating the
    attention output PSUM.
  - LN gamma/beta, softmax scale, and the V-bias are folded into the weight
    matrices host-side (exact algebraic rewrites).
"""

import numpy as np
import ml_dtypes

# ---- problem shapes (hardcoded; harness contract) ----
B, N, H = 16, 2048, 512
QS = 1024
NCORES = 8
BPC = B // NCORES          # 2 batches per core
EPS = 1e-5
P = 128
HCN = H // P               # 4 hidden chunks
H1CN = QS // P             # 8 mlp-hidden chunks
NT = N // P                # 16 token tiles
QBS = 512                  # query block size
NQB = N // QBS             # 4 query blocks
NKC = NT                   # 16 key chunks
GELU_SCALE = 1.702

# matmul operand dtypes per tensor ("bfloat16" or "float32" [runs as float32r])
DT_CFG = dict(
    z="bfloat16", qk="bfloat16", v="bfloat16", p="bfloat16",
    attn="bfloat16", h1="bfloat16", w="bfloat16", w1="bfloat16", w2="bfloat16",
)

LAST_RESULTS = None  # test.py introspection


def _np_dt(name):
    return ml_dtypes.bfloat16 if name == "bfloat16" else np.float32


def _build_program():
    import concourse.bass as bass
    import concourse.mybir as mybir
    import concourse.tile as tile
    from concourse import bacc
    from concourse.masks import make_identity

    dt = mybir.dt
    AF = mybir.ActivationFunctionType
    ALU = mybir.AluOpType

    def bdt(name):
        return dt.bfloat16 if name == "bfloat16" else dt.float32

    DZ, DQK, DV, DP = bdt(DT_CFG["z"]), bdt(DT_CFG["qk"]), bdt(DT_CFG["v"]), bdt(DT_CFG["p"])
    DA, DH1 = bdt(DT_CFG["attn"]), bdt(DT_CFG["h1"])
    DW, DW1, DW2 = bdt(DT_CFG["w"]), bdt(DT_CFG["w1"]), bdt(DT_CFG["w2"])
    F32 = dt.float32

    nc = bacc.Bacc("TRN2", target_bir_lowering=False)

    x_in = nc.dram_tensor("x", [BPC, N, H], F32, kind="ExternalInput")
    wq_d = nc.dram_tensor("wq", [H, H], DW, kind="ExternalInput")
    wk_d = nc.dram_tensor("wk", [H, H], DW, kind="ExternalInput")
    wv_d = nc.dram_tensor("wv", [H, H], DW, kind="ExternalInput")
    w1_d = nc.dram_tensor("w1", [H, QS], DW1, kind="ExternalInput")
    w2_d = nc.dram_tensor("w2m", [P, H1CN], DW2, kind="ExternalInput")
    bq_d = nc.dram_tensor("bq", [P, HCN], F32, kind="ExternalInput")
    bk_d = nc.dram_tensor("bk", [P, HCN], F32, kind="ExternalInput")
    b1a_d = nc.dram_tensor("b1a", [P, H1CN], F32, kind="ExternalInput")
    b1s_d = nc.dram_tensor("b1s", [P, H1CN], F32, kind="ExternalInput")
    b2_d = nc.dram_tensor("b2", [1, 1], F32, kind="ExternalInput")
    out_d = nc.dram_tensor("out", [BPC, N], F32, kind="ExternalOutput")

    def mm(out, lhsT, rhs, start, stop):
        # float32 operands run in float32r mode (1 cycle/row at N>=256)
        if lhsT.dtype == dt.float32:
            lhsT = lhsT.bitcast(dt.float32r)
        if rhs.dtype == dt.float32:
            rhs = rhs.bitcast(dt.float32r)
        nc.tensor.matmul(out, lhsT, rhs, start=start, stop=stop)

    with tile.TileContext(nc) as tc:
        with (
            tc.tile_pool(name="const", bufs=1) as cpool,
            tc.tile_pool(name="wpool", bufs=1) as wpool,
            tc.tile_pool(name="xin", bufs=6) as xpool,
            tc.tile_pool(name="stat", bufs=8) as spool,
            tc.tile_pool(name="big", bufs=1) as big,
            tc.tile_pool(name="work", bufs=2) as work,
            tc.tile_pool(name="ptp", bufs=4) as ptp,
            tc.tile_pool(name="psum", bufs=1, space="PSUM") as psum,
        ):
            # ---- constants ----
            ident = cpool.tile([P, P], F32, name="ident", tag="ident")
            make_identity(nc, ident)
            ones_col = cpool.tile([P, 1], DP, name="ones_col", tag="ones")
            nc.vector.memset(ones_col, 1.0)
            ones_mat = cpool.tile([P, P], DP, name="ones_mat", tag="onesm")
            nc.vector.memset(ones_mat, 1.0)
            eps_t = cpool.tile([P, 1], F32, name="eps_t", tag="eps")
            nc.vector.memset(eps_t, EPS)

            bq_sb = cpool.tile([P, HCN], F32, name="bq_sb", tag="bq")
            nc.sync.dma_start(out=bq_sb, in_=bq_d[:])
            bk_sb = cpool.tile([P, HCN], F32, name="bk_sb", tag="bk")
            nc.sync.dma_start(out=bk_sb, in_=bk_d[:])
            b1a_sb = cpool.tile([P, H1CN], F32, name="b1a_sb", tag="b1a")
            nc.sync.dma_start(out=b1a_sb, in_=b1a_d[:])
            b1s_sb = cpool.tile([P, H1CN], F32, name="b1s_sb", tag="b1s")
            nc.sync.dma_start(out=b1s_sb, in_=b1s_d[:])
            b2_sb = cpool.tile([1, 1], F32, name="b2_sb", tag="b2")
            nc.sync.dma_start(out=b2_sb, in_=b2_d[:])
            w2_sb = cpool.tile([P, H1CN], DW2, name="w2_sb", tag="w2")
            nc.sync.dma_start(out=w2_sb, in_=w2_d[:])

            # weights, chunk-major on partitions: w[p, c, j] = W[c*128+p, j]
            wq_sb = wpool.tile([P, HCN, H], DW, name="wq_sb", tag="wq")
            nc.sync.dma_start(out=wq_sb, in_=wq_d[:].rearrange("(c p) j -> p c j", p=P))
            wk_sb = wpool.tile([P, HCN, H], DW, name="wk_sb", tag="wk")
            nc.sync.dma_start(out=wk_sb, in_=wk_d[:].rearrange("(c p) j -> p c j", p=P))
            wv_sb = wpool.tile([P, HCN, H], DW, name="wv_sb", tag="wv")
            nc.sync.dma_start(out=wv_sb, in_=wv_d[:].rearrange("(c p) j -> p c j", p=P))
            w1_sb = wpool.tile([P, HCN, QS], DW1, name="w1_sb", tag="w1")
            nc.sync.dma_start(out=w1_sb, in_=w1_d[:].rearrange("(c p) j -> p c j", p=P))

            for b in range(BPC):
                # ---------- Phase 1: LayerNorm + transpose -> zT ----------
                zT = big.tile([P, HCN, N], DZ, name=f"zT_{b}", tag="zT")
                for tg in range(NT // 4):      # groups of 4 token tiles
                    xt = []
                    for i in range(4):
                        t = tg * 4 + i
                        x_t = xpool.tile([P, H], F32, name=f"x_{b}_{t}", tag="x")
                        nc.sync.dma_start(out=x_t, in_=x_in[b, t * P:(t + 1) * P, :])
                        stats = spool.tile([P, 6], F32, name=f"st_{b}_{t}", tag="st")
                        nc.vector.bn_stats(out=stats, in_=x_t)
                        mv = spool.tile([P, 2], F32, name=f"mv_{b}_{t}", tag="mv")
                        nc.vector.bn_aggr(out=mv, in_=stats)
                        sd = spool.tile([P, 1], F32, name=f"sd_{b}_{t}", tag="sd")
                        nc.scalar.activation(out=sd, in_=mv[:, 1:2], func=AF.Sqrt,
                                             bias=eps_t, scale=1.0)
                        rstd = spool.tile([P, 1], F32, name=f"rs_{b}_{t}", tag="rs")
                        nc.vector.reciprocal(out=rstd, in_=sd)
                        # x <- (x - mean) * rstd   (in place)
                        nc.vector.tensor_scalar(
                            out=x_t, in0=x_t, scalar1=mv[:, 0:1], scalar2=rstd,
                            op0=ALU.subtract, op1=ALU.mult)
                        xt.append(x_t)
                    for hc in range(HCN):
                        tp_ps = psum.tile([P, 512], F32, name=f"tp_{b}_{tg}_{hc}",
                                          tag="sc", bufs=2)
                        for i in range(4):
                            nc.tensor.transpose(
                                tp_ps[:, i * P:(i + 1) * P],
                                xt[i][:, hc * P:(hc + 1) * P], ident)
                        nc.scalar.copy(out=zT[:, hc, tg * 512:(tg + 1) * 512], in_=tp_ps)

                # ---------- Phase 2: Q^T, K^T, V ----------
                qT = big.tile([P, HCN, N], DQK, name=f"qT_{b}", tag="qT")
                kT = big.tile([P, HCN, N], DQK, name=f"kT_{b}", tag="kT")
                vN = big.tile([P, NT, H], DV, name=f"vN_{b}", tag="vN")
                for ho in range(HCN):
                    for tq in range(N // 512):
                        q_ps = psum.tile([P, 512], F32, name=f"q_{b}_{ho}_{tq}",
                                         tag="sc", bufs=2)
                        for hc in range(HCN):
                            mm(q_ps, wq_sb[:, hc, ho * P:(ho + 1) * P],
                               zT[:, hc, tq * 512:(tq + 1) * 512],
                               start=(hc == 0), stop=(hc == HCN - 1))
                        nc.scalar.activation(
                            out=qT[:, ho, tq * 512:(tq + 1) * 512], in_=q_ps,
                            func=AF.Identity, bias=bq_sb[:, ho:ho + 1], scale=1.0)
                        k_ps = psum.tile([P, 512], F32, name=f"k_{b}_{ho}_{tq}",
                                         tag="sc", bufs=2)
                        for hc in range(HCN):
                            mm(k_ps, wk_sb[:, hc, ho * P:(ho + 1) * P],
                               zT[:, hc, tq * 512:(tq + 1) * 512],
                               start=(hc == 0), stop=(hc == HCN - 1))
                        nc.scalar.activation(
                            out=kT[:, ho, tq * 512:(tq + 1) * 512], in_=k_ps,
                            func=AF.Identity, bias=bk_sb[:, ho:ho + 1], scale=1.0)
                for tv in range(NT):
                    v_ps = psum.tile([P, H], F32, name=f"v_{b}_{tv}", tag="sc", bufs=2)
                    for hc in range(HCN):
                        mm(v_ps, zT[:, hc, tv * P:(tv + 1) * P], wv_sb[:, hc, :],
                           start=(hc == 0), stop=(hc == HCN - 1))
                    nc.vector.tensor_copy(out=vN[:, tv, :], in_=v_ps)

                # ---------- Phase 3: attention + MLP per query block ----------
                out_row_parts = []
                for qb in range(NQB):
                    qsl = slice(qb * QBS, (qb + 1) * QBS)
                    attn_ps = [psum.tile([P, QBS], F32, name=f"ap_{b}_{qb}_{hc}",
                                         tag=f"a{hc}", bufs=1) for hc in range(HCN)]
                    row_ps = psum.tile([P, QBS], F32, name=f"row_{b}_{qb}",
                                       tag="row", bufs=1)
                    for kc in range(NKC):
                        sc_ps = psum.tile([P, QBS], F32, name=f"sc_{b}_{qb}_{kc}",
                                          tag="sc", bufs=2)
                        for hc in range(HCN):
                            mm(sc_ps, kT[:, hc, kc * P:(kc + 1) * P], qT[:, hc, qsl],
                               start=(hc == 0), stop=(hc == HCN - 1))
                        pt = ptp.tile([P, QBS], DP, name=f"pt_{b}_{qb}_{kc}", tag="pt")
                        nc.scalar.activation(out=pt, in_=sc_ps, func=AF.Exp,
                                             bias=0.0, scale=1.0)
                        mm(row_ps, ones_mat, pt, start=(kc == 0), stop=(kc == NKC - 1))
                        for hc in range(HCN):
                            mm(attn_ps[hc], vN[:, kc, hc * P:(hc + 1) * P], pt,
                               start=(kc == 0), stop=(kc == NKC - 1))
                    # rowsum is replicated on all 128 partitions (ones-matrix lhsT)
                    rb = work.tile([P, QBS], F32, name=f"rb_{b}_{qb}", tag="rb")
                    nc.vector.reciprocal(out=rb, in_=row_ps)
                    attn_sb = work.tile([P, HCN, QBS], DA, name=f"at_{b}_{qb}", tag="at")
                    for hc in range(HCN):
                        nc.vector.tensor_tensor(
                            out=attn_sb[:, hc, :], in0=attn_ps[hc], in1=rb,
                            op=ALU.mult)
                    # MLP1 + quick-GELU
                    h1_sb = work.tile([P, H1CN, QBS], DH1, name=f"h1_{b}_{qb}", tag="h1")
                    for c1 in range(H1CN):
                        u_ps = psum.tile([P, QBS], F32, name=f"u_{b}_{qb}_{c1}",
                                         tag=f"a{c1 % HCN}", bufs=1)
                        for hc in range(HCN):
                            mm(u_ps, w1_sb[:, hc, c1 * P:(c1 + 1) * P],
                               attn_sb[:, hc, :],
                               start=(hc == 0), stop=(hc == HCN - 1))
                        sig = work.tile([P, QBS], DH1, name=f"sg_{b}_{qb}_{c1}", tag="sg")
                        nc.scalar.activation(out=sig, in_=u_ps, func=AF.Sigmoid,
                                             bias=b1s_sb[:, c1:c1 + 1], scale=GELU_SCALE)
                        ab = work.tile([P, QBS], DH1, name=f"ab_{b}_{qb}_{c1}", tag="ab")
                        nc.vector.tensor_scalar_add(out=ab, in0=u_ps,
                                                    scalar1=b1a_sb[:, c1:c1 + 1])
                        nc.vector.tensor_tensor(out=h1_sb[:, c1, :], in0=ab, in1=sig,
                                                op=ALU.mult)
                    # MLP2
                    o_ps = psum.tile([1, QBS], F32, name=f"o_{b}_{qb}", tag="o", bufs=1)
                    for c1 in range(H1CN):
                        mm(o_ps, w2_sb[:, c1:c1 + 1], h1_sb[:, c1, :],
                           start=(c1 == 0), stop=(c1 == H1CN - 1))
                    orow = work.tile([1, QBS], F32, name=f"or_{b}_{qb}", tag="or")
                    nc.scalar.activation(out=orow, in_=o_ps, func=AF.Identity,
                                         bias=b2_sb[0:1, 0:1], scale=1.0)
                    out_row_parts.append(orow)
                    nc.sync.dma_start(out=out_d[b:b + 1, qsl], in_=orow)

    nc.finalize()
    return nc


def _prep_inputs(inputs):
    """Fold LN affine, softmax scale, and V-bias into weights (exact rewrites)."""
    f32 = np.float32
    x = np.ascontiguousarray(np.asarray(inputs["x"], dtype=f32))
    g = np.asarray(inputs["ln_g"], dtype=f32)
    bb = np.asarray(inputs["ln_b"], dtype=f32)
    Wq = np.asarray(inputs["Wq"], dtype=f32)
    Wk = np.asarray(inputs["Wk"], dtype=f32)
    Wv = np.asarray(inputs["Wv"], dtype=f32)
    bq = np.asarray(inputs["bq"], dtype=f32)
    bk = np.asarray(inputs["bk"], dtype=f32)
    bv = np.asarray(inputs["bv"], dtype=f32)
    W1 = np.asarray(inputs["W1"], dtype=f32)
    b1 = np.asarray(inputs["b1"], dtype=f32)
    W2 = np.asarray(inputs["W2"], dtype=f32)
    b2 = np.asarray(inputs["b2"], dtype=f32)

    s = f32(1.0 / np.sqrt(H))
    Wq2 = (g[:, None] * Wq) * s
    bq2 = (bb @ Wq + bq) * s
    Wk2 = g[:, None] * Wk
    bk2 = bb @ Wk + bk
    Wv2 = g[:, None] * Wv
    bv2 = bb @ Wv + bv
    b1f = b1 + bv2 @ W1          # V-bias folded through MLP1 (softmax rows sum to 1)
    b1s = f32(GELU_SCALE) * b1f

    def cm(v, n):                # [n*128] -> [128, n] chunk-major columns
        return np.ascontiguousarray(v.reshape(n, P).T)

    feed = dict(
        wq=Wq2.astype(_np_dt(DT_CFG["w"])),
        wk=Wk2.astype(_np_dt(DT_CFG["w"])),
        wv=Wv2.astype(_np_dt(DT_CFG["w"])),
        w1=W1.astype(_np_dt(DT_CFG["w1"])),
        w2m=cm(W2[:, 0], H1CN).astype(_np_dt(DT_CFG["w2"])),
        bq=cm(bq2, HCN).astype(f32),
        bk=cm(bk2, HCN).astype(f32),
        b1a=cm(b1f, H1CN).astype(f32),
        b1s=cm(b1s, H1CN).astype(f32),
        b2=b2.reshape(1, 1).astype(f32),
    )
    return x, feed


def _bench(inputs, iters=20):
    """Correctness + steady-state timing: one compile, repeated execution with
    inputs resident on device. Returns (out[16,2048], per_iter_seconds)."""
    import time
    import jax
    from jax.experimental.shard_map import shard_map
    from jax.sharding import Mesh, NamedSharding, PartitionSpec
    from concourse import bass2jax, mybir

    x, feed = _prep_inputs(inputs)
    nc = _build_program()
    bass2jax.install_neuronx_cc_hook()

    partition_name = nc.partition_id_tensor.name if nc.partition_id_tensor else None
    in_names, out_names, out_avals, zero_outs = [], [], [], []
    for alloc in nc.m.functions[0].allocations:
        if not isinstance(alloc, mybir.MemoryLocationSet):
            continue
        name = alloc.memorylocations[0].name
        if alloc.kind == "ExternalInput":
            if name != partition_name:
                in_names.append(name)
        elif alloc.kind == "ExternalOutput":
            shape = tuple(alloc.tensor_shape)
            dtype = mybir.dt.np(alloc.dtype)
            out_names.append(name)
            out_avals.append(jax.core.ShapedArray(shape, dtype))
            zero_outs.append(np.zeros(shape, dtype))
    n_params = len(in_names)
    all_in_names = list(in_names) + list(out_names)
    if partition_name is not None:
        all_in_names.append(partition_name)

    def _body(*args):
        operands = list(args)
        if partition_name is not None:
            operands.append(bass2jax.partition_id_tensor())
        outs = bass2jax._bass_exec_p.bind(
            *operands,
            out_avals=tuple(out_avals),
            in_names=tuple(all_in_names),
            out_names=tuple(out_names),
            lowering_input_output_aliases=(),
            sim_require_finite=True,
            sim_require_nnan=True,
            nc=nc,
        )
        return tuple(outs)

    devices = jax.devices()[:NCORES]
    mesh = Mesh(np.asarray(devices), ("core",))
    n_outs = len(out_names)
    in_specs = (PartitionSpec("core"),) * (n_params + n_outs)
    out_specs = (PartitionSpec("core"),) * n_outs
    sharded = jax.jit(shard_map(_body, mesh=mesh, in_specs=in_specs,
                                out_specs=out_specs, check_rep=False),
                      keep_unused=True)

    in_maps = []
    for c in range(NCORES):
        m = dict(feed)
        m["x"] = np.ascontiguousarray(x[c * BPC:(c + 1) * BPC])
        in_maps.append(m)
    per_core = [[np.asarray(m[nm]) for nm in in_names] for m in in_maps]
    concat_in = [np.concatenate([per_core[c][i] for c in range(NCORES)], axis=0)
                 for i in range(n_params)]
    concat_zero = [np.zeros((NCORES * z.shape[0], *z.shape[1:]), z.dtype)
                   for z in zero_outs]
    sh = NamedSharding(mesh, PartitionSpec("core"))
    dev_in = [jax.device_put(a, sh) for a in concat_in + concat_zero]

    out_arrs = sharded(*dev_in)           # compile + first exec
    jax.block_until_ready(out_arrs)
    oi = out_names.index("out")
    out = np.asarray(out_arrs[oi]).reshape(B, N).astype(np.float32)

    t0 = time.time()
    for _ in range(iters):
        r = sharded(*dev_in)
    jax.block_until_ready(r)
    per_iter = (time.time() - t0) / iters
    return out, per_iter


def _run(inputs, trace=False, **spmd_kwargs):
    global LAST_RESULTS
    from concourse.bass_utils import run_bass_kernel_spmd

    x, feed = _prep_inputs(inputs)
    nc = _build_program()
    in_maps = []
    for c in range(NCORES):
        m = dict(feed)
        m["x"] = np.ascontiguousarray(x[c * BPC:(c + 1) * BPC])
        in_maps.append(m)
    res = run_bass_kernel_spmd(nc, in_maps, core_ids=list(range(NCORES)),
                               trace=trace, **spmd_kwargs)
    LAST_RESULTS = res
    out = np.concatenate([r["out"] for r in res.results], axis=0)
    return np.ascontiguousarray(out.astype(np.float32))


def kernel(**inputs):
    return _run(inputs, trace=False)


# revision 20
# speedup vs baseline: 20.1444x; 20.1444x over previous
"""Fused dense-transformer block for Trainium2 (Bass/Tile), 8-core data-parallel.

Per batch row b of x[16, 2048, 512]:
  LayerNorm -> Q/K/V proj -> softmax(Q K^T / sqrt(H)) V -> quickGELU MLP(512->1024->1) -> [2048]

Sharding: batch dim 16 -> 8 cores x 2 batches each. No collectives.

Layout strategy (per core, per batch):
  - LN in natural layout [tok, h] (free-dim stats via bn_stats), then PE-transpose
    the normalized activations to z^T [h, tok].
  - Q^T, K^T computed in transposed layout [h_out, tok]; V in natural [tok, h].
  - Scores computed directly TRANSPOSED: S^T[key, q] = (K z-weights) so softmax
    exp happens on ACT reading PSUM, writing P^T[key, q] straight to SBUF --
    no transposes of the big [2048, 2048] attention matrix.
  - Row sums via ones-vector matmul (contraction over keys = partitions),
    reciprocal on DVE, broadcast via gpsimd, applied while evacuating the
    attention output PSUM.
  - LN gamma/beta, softmax scale, and the V-bias are folded into the weight
    matrices host-side (exact algebraic rewrites).
"""

import numpy as np
import ml_dtypes

# ---- problem shapes (hardcoded; harness contract) ----
B, N, H = 16, 2048, 512
QS = 1024
NCORES = 8
BPC = B // NCORES          # 2 batches per core
EPS = 1e-5
P = 128
HCN = H // P               # 4 hidden chunks
H1CN = QS // P             # 8 mlp-hidden chunks
NT = N // P                # 16 token tiles
QBS = 512                  # query block size
NQB = N // QBS             # 4 query blocks
NKC = NT                   # 16 key chunks
GELU_SCALE = 1.702

# matmul operand dtypes per tensor ("bfloat16" or "float32" [runs as float32r])
DT_CFG = dict(
    z="bfloat16", qk="bfloat16", v="bfloat16", p="bfloat16",
    attn="bfloat16", h1="bfloat16", w="bfloat16", w1="bfloat16", w2="bfloat16",
)

LAST_RESULTS = None  # test.py introspection


def _np_dt(name):
    return ml_dtypes.bfloat16 if name == "bfloat16" else np.float32


def _build_program(reps=1):
    from contextlib import ExitStack

    import concourse.bass as bass
    import concourse.mybir as mybir
    import concourse.tile as tile
    from concourse import bacc
    from concourse.masks import make_identity

    dt = mybir.dt
    AF = mybir.ActivationFunctionType
    ALU = mybir.AluOpType

    def bdt(name):
        return dt.bfloat16 if name == "bfloat16" else dt.float32

    DZ, DQK, DV, DP = bdt(DT_CFG["z"]), bdt(DT_CFG["qk"]), bdt(DT_CFG["v"]), bdt(DT_CFG["p"])
    DA, DH1 = bdt(DT_CFG["attn"]), bdt(DT_CFG["h1"])
    DW, DW1, DW2 = bdt(DT_CFG["w"]), bdt(DT_CFG["w1"]), bdt(DT_CFG["w2"])
    F32 = dt.float32

    nc = bacc.Bacc("TRN2", target_bir_lowering=False)

    x_in = nc.dram_tensor("x", [BPC, N, H], F32, kind="ExternalInput")
    wq_d = nc.dram_tensor("wq", [H, H], DW, kind="ExternalInput")
    wk_d = nc.dram_tensor("wk", [H, H], DW, kind="ExternalInput")
    wv_d = nc.dram_tensor("wv", [H, H], DW, kind="ExternalInput")
    w1_d = nc.dram_tensor("w1", [H, QS], DW1, kind="ExternalInput")
    w2_d = nc.dram_tensor("w2m", [P, H1CN], DW2, kind="ExternalInput")
    bq_d = nc.dram_tensor("bq", [P, HCN], F32, kind="ExternalInput")
    bk_d = nc.dram_tensor("bk", [P, HCN], F32, kind="ExternalInput")
    b1a_d = nc.dram_tensor("b1a", [P, H1CN], F32, kind="ExternalInput")
    b1s_d = nc.dram_tensor("b1s", [P, H1CN], F32, kind="ExternalInput")
    b2_d = nc.dram_tensor("b2", [1, 1], F32, kind="ExternalInput")
    out_d = nc.dram_tensor("out", [BPC, N], F32, kind="ExternalOutput")

    def mm(out, lhsT, rhs, start, stop):
        # float32 operands run in float32r mode (1 cycle/row at N>=256)
        if lhsT.dtype == dt.float32:
            lhsT = lhsT.bitcast(dt.float32r)
        if rhs.dtype == dt.float32:
            rhs = rhs.bitcast(dt.float32r)
        nc.tensor.matmul(out, lhsT, rhs, start=start, stop=stop)

    with tile.TileContext(nc) as tc:
        with (
            tc.tile_pool(name="const", bufs=1) as cpool,
            tc.tile_pool(name="wpool", bufs=1) as wpool,
            tc.tile_pool(name="xin", bufs=6) as xpool,
            tc.tile_pool(name="stat", bufs=8) as spool,
            tc.tile_pool(name="big", bufs=1) as big,
            tc.tile_pool(name="work", bufs=2) as work,
            tc.tile_pool(name="ptp", bufs=4) as ptp,
            tc.tile_pool(name="psum", bufs=1, space="PSUM") as psum,
        ):
            # ---- constants ----
            ident = cpool.tile([P, P], F32, name="ident", tag="ident")
            make_identity(nc, ident)
            ones_col = cpool.tile([P, 1], DP, name="ones_col", tag="ones")
            nc.vector.memset(ones_col, 1.0)
            ones_mat = cpool.tile([P, P], DP, name="ones_mat", tag="onesm")
            nc.vector.memset(ones_mat, 1.0)
            eps_t = cpool.tile([P, 1], F32, name="eps_t", tag="eps")
            nc.vector.memset(eps_t, EPS)

            bq_sb = cpool.tile([P, HCN], F32, name="bq_sb", tag="bq")
            nc.sync.dma_start(out=bq_sb, in_=bq_d[:])
            bk_sb = cpool.tile([P, HCN], F32, name="bk_sb", tag="bk")
            nc.sync.dma_start(out=bk_sb, in_=bk_d[:])
            b1a_sb = cpool.tile([P, H1CN], F32, name="b1a_sb", tag="b1a")
            nc.sync.dma_start(out=b1a_sb, in_=b1a_d[:])
            b1s_sb = cpool.tile([P, H1CN], F32, name="b1s_sb", tag="b1s")
            nc.sync.dma_start(out=b1s_sb, in_=b1s_d[:])
            b2_sb = cpool.tile([1, 1], F32, name="b2_sb", tag="b2")
            nc.sync.dma_start(out=b2_sb, in_=b2_d[:])
            w2_sb = cpool.tile([P, H1CN], DW2, name="w2_sb", tag="w2")
            nc.sync.dma_start(out=w2_sb, in_=w2_d[:])

            # weights, chunk-major on partitions: w[p, c, j] = W[c*128+p, j]
            wq_sb = wpool.tile([P, HCN, H], DW, name="wq_sb", tag="wq")
            nc.sync.dma_start(out=wq_sb, in_=wq_d[:].rearrange("(c p) j -> p c j", p=P))
            wk_sb = wpool.tile([P, HCN, H], DW, name="wk_sb", tag="wk")
            nc.sync.dma_start(out=wk_sb, in_=wk_d[:].rearrange("(c p) j -> p c j", p=P))
            wv_sb = wpool.tile([P, HCN, H], DW, name="wv_sb", tag="wv")
            nc.sync.dma_start(out=wv_sb, in_=wv_d[:].rearrange("(c p) j -> p c j", p=P))
            w1_sb = wpool.tile([P, HCN, QS], DW1, name="w1_sb", tag="w1")
            nc.sync.dma_start(out=w1_sb, in_=w1_d[:].rearrange("(c p) j -> p c j", p=P))

            rep_ctx = ExitStack()
            if reps > 1:
                # benchmark-only: repeat the whole body in a HW loop so device
                # time can be measured as a slope over reps (cancels dispatch
                # overhead). reps=1 (graded path) emits no loop at all.
                rep_ctx.enter_context(tc.For_i(0, reps, 1))
            for b in range(BPC):
                # ---------- Phase 1: LayerNorm + transpose -> zT ----------
                zT = big.tile([P, HCN, N], DZ, name=f"zT_{b}", tag="zT")
                for tg in range(NT // 4):      # groups of 4 token tiles
                    xt = []
                    for i in range(4):
                        t = tg * 4 + i
                        x_t = xpool.tile([P, H], F32, name=f"x_{b}_{t}", tag="x")
                        nc.sync.dma_start(out=x_t, in_=x_in[b, t * P:(t + 1) * P, :])
                        stats = spool.tile([P, 6], F32, name=f"st_{b}_{t}", tag="st")
                        nc.vector.bn_stats(out=stats, in_=x_t)
                        mv = spool.tile([P, 2], F32, name=f"mv_{b}_{t}", tag="mv")
                        nc.vector.bn_aggr(out=mv, in_=stats)
                        sd = spool.tile([P, 1], F32, name=f"sd_{b}_{t}", tag="sd")
                        nc.scalar.activation(out=sd, in_=mv[:, 1:2], func=AF.Sqrt,
                                             bias=eps_t, scale=1.0)
                        rstd = spool.tile([P, 1], F32, name=f"rs_{b}_{t}", tag="rs")
                        nc.vector.reciprocal(out=rstd, in_=sd)
                        # x <- (x - mean) * rstd   (in place)
                        nc.vector.tensor_scalar(
                            out=x_t, in0=x_t, scalar1=mv[:, 0:1], scalar2=rstd,
                            op0=ALU.subtract, op1=ALU.mult)
                        xt.append(x_t)
                    for hc in range(HCN):
                        tp_ps = psum.tile([P, 512], F32, name=f"tp_{b}_{tg}_{hc}",
                                          tag="sc", bufs=2)
                        for i in range(4):
                            nc.tensor.transpose(
                                tp_ps[:, i * P:(i + 1) * P],
                                xt[i][:, hc * P:(hc + 1) * P], ident)
                        nc.scalar.copy(out=zT[:, hc, tg * 512:(tg + 1) * 512], in_=tp_ps)

                # ---------- Phase 2: Q^T, K^T, V ----------
                qT = big.tile([P, HCN, N], DQK, name=f"qT_{b}", tag="qT")
                kT = big.tile([P, HCN, N], DQK, name=f"kT_{b}", tag="kT")
                vN = big.tile([P, NT, H], DV, name=f"vN_{b}", tag="vN")
                for ho in range(HCN):
                    for tq in range(N // 512):
                        q_ps = psum.tile([P, 512], F32, name=f"q_{b}_{ho}_{tq}",
                                         tag="sc", bufs=2)
                        for hc in range(HCN):
                            mm(q_ps, wq_sb[:, hc, ho * P:(ho + 1) * P],
                               zT[:, hc, tq * 512:(tq + 1) * 512],
                               start=(hc == 0), stop=(hc == HCN - 1))
                        nc.scalar.activation(
                            out=qT[:, ho, tq * 512:(tq + 1) * 512], in_=q_ps,
                            func=AF.Identity, bias=bq_sb[:, ho:ho + 1], scale=1.0)
                        k_ps = psum.tile([P, 512], F32, name=f"k_{b}_{ho}_{tq}",
                                         tag="sc", bufs=2)
                        for hc in range(HCN):
                            mm(k_ps, wk_sb[:, hc, ho * P:(ho + 1) * P],
                               zT[:, hc, tq * 512:(tq + 1) * 512],
                               start=(hc == 0), stop=(hc == HCN - 1))
                        nc.scalar.activation(
                            out=kT[:, ho, tq * 512:(tq + 1) * 512], in_=k_ps,
                            func=AF.Identity, bias=bk_sb[:, ho:ho + 1], scale=1.0)
                for tv in range(NT):
                    v_ps = psum.tile([P, H], F32, name=f"v_{b}_{tv}", tag="sc", bufs=2)
                    for hc in range(HCN):
                        mm(v_ps, zT[:, hc, tv * P:(tv + 1) * P], wv_sb[:, hc, :],
                           start=(hc == 0), stop=(hc == HCN - 1))
                    nc.vector.tensor_copy(out=vN[:, tv, :], in_=v_ps)

                # ---------- Phase 3: attention + MLP per query block ----------
                out_row_parts = []
                for qb in range(NQB):
                    qsl = slice(qb * QBS, (qb + 1) * QBS)
                    attn_ps = [psum.tile([P, QBS], F32, name=f"ap_{b}_{qb}_{hc}",
                                         tag=f"a{hc}", bufs=1) for hc in range(HCN)]
                    row_ps = psum.tile([P, QBS], F32, name=f"row_{b}_{qb}",
                                       tag="row", bufs=1)
                    for kc in range(NKC):
                        sc_ps = psum.tile([P, QBS], F32, name=f"sc_{b}_{qb}_{kc}",
                                          tag="sc", bufs=2)
                        for hc in range(HCN):
                            mm(sc_ps, kT[:, hc, kc * P:(kc + 1) * P], qT[:, hc, qsl],
                               start=(hc == 0), stop=(hc == HCN - 1))
                        pt = ptp.tile([P, QBS], DP, name=f"pt_{b}_{qb}_{kc}", tag="pt")
                        nc.scalar.activation(out=pt, in_=sc_ps, func=AF.Exp,
                                             bias=0.0, scale=1.0)
                        mm(row_ps, ones_mat, pt, start=(kc == 0), stop=(kc == NKC - 1))
                        for hc in range(HCN):
                            mm(attn_ps[hc], vN[:, kc, hc * P:(hc + 1) * P], pt,
                               start=(kc == 0), stop=(kc == NKC - 1))
                    # rowsum is replicated on all 128 partitions (ones-matrix lhsT)
                    rb = work.tile([P, QBS], F32, name=f"rb_{b}_{qb}", tag="rb")
                    nc.vector.reciprocal(out=rb, in_=row_ps)
                    attn_sb = work.tile([P, HCN, QBS], DA, name=f"at_{b}_{qb}", tag="at")
                    for hc in range(HCN):
                        nc.vector.tensor_tensor(
                            out=attn_sb[:, hc, :], in0=attn_ps[hc], in1=rb,
                            op=ALU.mult)
                    # MLP1 + quick-GELU
                    h1_sb = work.tile([P, H1CN, QBS], DH1, name=f"h1_{b}_{qb}", tag="h1")
                    for c1 in range(H1CN):
                        u_ps = psum.tile([P, QBS], F32, name=f"u_{b}_{qb}_{c1}",
                                         tag=f"a{c1 % HCN}", bufs=1)
                        for hc in range(HCN):
                            mm(u_ps, w1_sb[:, hc, c1 * P:(c1 + 1) * P],
                               attn_sb[:, hc, :],
                               start=(hc == 0), stop=(hc == HCN - 1))
                        sig = work.tile([P, QBS], DH1, name=f"sg_{b}_{qb}_{c1}", tag="sg")
                        nc.scalar.activation(out=sig, in_=u_ps, func=AF.Sigmoid,
                                             bias=b1s_sb[:, c1:c1 + 1], scale=GELU_SCALE)
                        ab = work.tile([P, QBS], DH1, name=f"ab_{b}_{qb}_{c1}", tag="ab")
                        nc.vector.tensor_scalar_add(out=ab, in0=u_ps,
                                                    scalar1=b1a_sb[:, c1:c1 + 1])
                        nc.vector.tensor_tensor(out=h1_sb[:, c1, :], in0=ab, in1=sig,
                                                op=ALU.mult)
                    # MLP2
                    o_ps = psum.tile([1, QBS], F32, name=f"o_{b}_{qb}", tag="o", bufs=1)
                    for c1 in range(H1CN):
                        mm(o_ps, w2_sb[:, c1:c1 + 1], h1_sb[:, c1, :],
                           start=(c1 == 0), stop=(c1 == H1CN - 1))
                    orow = work.tile([1, QBS], F32, name=f"or_{b}_{qb}", tag="or")
                    nc.scalar.activation(out=orow, in_=o_ps, func=AF.Identity,
                                         bias=b2_sb[0:1, 0:1], scale=1.0)
                    out_row_parts.append(orow)
                    nc.sync.dma_start(out=out_d[b:b + 1, qsl], in_=orow)

            rep_ctx.close()

    nc.finalize()
    return nc


def _prep_inputs(inputs):
    """Fold LN affine, softmax scale, and V-bias into weights (exact rewrites)."""
    f32 = np.float32
    x = np.ascontiguousarray(np.asarray(inputs["x"], dtype=f32))
    g = np.asarray(inputs["ln_g"], dtype=f32)
    bb = np.asarray(inputs["ln_b"], dtype=f32)
    Wq = np.asarray(inputs["Wq"], dtype=f32)
    Wk = np.asarray(inputs["Wk"], dtype=f32)
    Wv = np.asarray(inputs["Wv"], dtype=f32)
    bq = np.asarray(inputs["bq"], dtype=f32)
    bk = np.asarray(inputs["bk"], dtype=f32)
    bv = np.asarray(inputs["bv"], dtype=f32)
    W1 = np.asarray(inputs["W1"], dtype=f32)
    b1 = np.asarray(inputs["b1"], dtype=f32)
    W2 = np.asarray(inputs["W2"], dtype=f32)
    b2 = np.asarray(inputs["b2"], dtype=f32)

    s = f32(1.0 / np.sqrt(H))
    Wq2 = (g[:, None] * Wq) * s
    bq2 = (bb @ Wq + bq) * s
    Wk2 = g[:, None] * Wk
    bk2 = bb @ Wk + bk
    Wv2 = g[:, None] * Wv
    bv2 = bb @ Wv + bv
    b1f = b1 + bv2 @ W1          # V-bias folded through MLP1 (softmax rows sum to 1)
    b1s = f32(GELU_SCALE) * b1f

    def cm(v, n):                # [n*128] -> [128, n] chunk-major columns
        return np.ascontiguousarray(v.reshape(n, P).T)

    feed = dict(
        wq=Wq2.astype(_np_dt(DT_CFG["w"])),
        wk=Wk2.astype(_np_dt(DT_CFG["w"])),
        wv=Wv2.astype(_np_dt(DT_CFG["w"])),
        w1=W1.astype(_np_dt(DT_CFG["w1"])),
        w2m=cm(W2[:, 0], H1CN).astype(_np_dt(DT_CFG["w2"])),
        bq=cm(bq2, HCN).astype(f32),
        bk=cm(bk2, HCN).astype(f32),
        b1a=cm(b1f, H1CN).astype(f32),
        b1s=cm(b1s, H1CN).astype(f32),
        b2=b2.reshape(1, 1).astype(f32),
    )
    return x, feed


def _bench(inputs, iters=20, inner=1, reps=1):
    """Correctness + steady-state timing: one compile, repeated execution with
    inputs resident on device. `reps` repeats the kernel body in a HW loop so
    device time can be measured as a slope over reps. Returns (out, per_iter_s)."""
    import time
    import jax
    from jax.experimental.shard_map import shard_map
    from jax.sharding import Mesh, NamedSharding, PartitionSpec
    from concourse import bass2jax, mybir

    x, feed = _prep_inputs(inputs)
    nc = _build_program(reps=reps)
    bass2jax.install_neuronx_cc_hook()

    partition_name = nc.partition_id_tensor.name if nc.partition_id_tensor else None
    in_names, out_names, out_avals, zero_outs = [], [], [], []
    for alloc in nc.m.functions[0].allocations:
        if not isinstance(alloc, mybir.MemoryLocationSet):
            continue
        name = alloc.memorylocations[0].name
        if alloc.kind == "ExternalInput":
            if name != partition_name:
                in_names.append(name)
        elif alloc.kind == "ExternalOutput":
            shape = tuple(alloc.tensor_shape)
            dtype = mybir.dt.np(alloc.dtype)
            out_names.append(name)
            out_avals.append(jax.core.ShapedArray(shape, dtype))
            zero_outs.append(np.zeros(shape, dtype))
    n_params = len(in_names)
    all_in_names = list(in_names) + list(out_names)
    if partition_name is not None:
        all_in_names.append(partition_name)

    def _body(*args):
        ins = list(args[:n_params])
        outs = list(args[n_params:])
        for _ in range(inner):
            operands = ins + outs
            if partition_name is not None:
                operands.append(bass2jax.partition_id_tensor())
            outs = list(bass2jax._bass_exec_p.bind(
                *operands,
                out_avals=tuple(out_avals),
                in_names=tuple(all_in_names),
                out_names=tuple(out_names),
                lowering_input_output_aliases=(),
                sim_require_finite=True,
                sim_require_nnan=True,
                nc=nc,
            ))
        return tuple(outs)

    devices = jax.devices()[:NCORES]
    mesh = Mesh(np.asarray(devices), ("core",))
    n_outs = len(out_names)
    in_specs = (PartitionSpec("core"),) * (n_params + n_outs)
    out_specs = (PartitionSpec("core"),) * n_outs
    sharded = jax.jit(shard_map(_body, mesh=mesh, in_specs=in_specs,
                                out_specs=out_specs, check_rep=False),
                      keep_unused=True)

    in_maps = []
    for c in range(NCORES):
        m = dict(feed)
        m["x"] = np.ascontiguousarray(x[c * BPC:(c + 1) * BPC])
        in_maps.append(m)
    per_core = [[np.asarray(m[nm]) for nm in in_names] for m in in_maps]
    concat_in = [np.concatenate([per_core[c][i] for c in range(NCORES)], axis=0)
                 for i in range(n_params)]
    concat_zero = [np.zeros((NCORES * z.shape[0], *z.shape[1:]), z.dtype)
                   for z in zero_outs]
    sh = NamedSharding(mesh, PartitionSpec("core"))
    dev_in = [jax.device_put(a, sh) for a in concat_in + concat_zero]

    out_arrs = sharded(*dev_in)           # compile + first exec
    jax.block_until_ready(out_arrs)
    oi = out_names.index("out")
    out = np.asarray(out_arrs[oi]).reshape(B, N).astype(np.float32)

    t0 = time.time()
    for _ in range(iters):
        r = sharded(*dev_in)
    jax.block_until_ready(r)
    per_iter = (time.time() - t0) / iters
    return out, per_iter


def _run(inputs, trace=False, **spmd_kwargs):
    global LAST_RESULTS
    from concourse.bass_utils import run_bass_kernel_spmd

    x, feed = _prep_inputs(inputs)
    nc = _build_program()
    in_maps = []
    for c in range(NCORES):
        m = dict(feed)
        m["x"] = np.ascontiguousarray(x[c * BPC:(c + 1) * BPC])
        in_maps.append(m)
    res = run_bass_kernel_spmd(nc, in_maps, core_ids=list(range(NCORES)),
                               trace=trace, **spmd_kwargs)
    LAST_RESULTS = res
    out = np.concatenate([r["out"] for r in res.results], axis=0)
    return np.ascontiguousarray(out.astype(np.float32))


def kernel(**inputs):
    return _run(inputs, trace=False)
